# revision 2
# baseline (speedup 1.0000x reference)
"""Connectome kernel (segment-mean -> Pearson Gram) for 8 TRN2 NeuronCores.

Strategy (pure data parallel, 2 samples per core):
  - Host marshalling: fold mask into parcellation; DROP background /
    masked-out pixels (~50% of V) entirely; sort surviving pixels by ROI
    and pack them into 128-pixel chunks (block A = ROIs 0..127, block B =
    ROIs 128..199, each padded to a chunk boundary with label -1 slots).
    x is gathered into this packed order, cast fp16, and laid out
    [p, chunk, sample, t] per core so each SBUF partition reads one
    contiguous HBM run per chunk-tile. Wire traffic per core drops from
    73.7MB (fp32, all pixels) to ~18.3MB.
  - Device: stream chunk-tiles (2 DMA queues alternating); per chunk
    build a onehot (128, width) on DVE from the packed labels (is_equal
    vs iota) and accumulate roiT[r, row] += onehot.T @ x_chunk on PE
    (fp16 operands, fp32 PSUM). Sorted packing means each chunk feeds
    exactly one ROI block -> one matmul per chunk, 400 moving cols.
  - Epilogue per core (fp32): scale by 1/count, demean over t, normalize
    (rsqrt), transpose (PE), Gram matmul, write (2,200,200) conn to HBM.
  - Host: concat cores, extract upper triangle -> (16, 19900).
"""
import sys

sys.path.insert(0, "/opt/trn_rl_repo")

import numpy as np

import concourse.bass as bass
import concourse.tile as tile
from concourse import bacc, mybir
from concourse.bass_utils import run_bass_kernel_spmd

F32 = mybir.dt.float32
F16 = mybir.dt.float16

N, T, H, W = 16, 200, 144, 320
V = H * W                      # 46080
R = 200                        # ROIs
RA = 128                       # ROI block A width
RB = R - RA                    # ROI block B width (72)
NCORES = 8
SPB = N // NCORES              # samples per core = 2
ROWS = SPB * T                 # 400
CT = 32                        # chunks per DMA tile
EPS = 1e-8

_cached = {}


def _build_program(nA, nB):
    nch = nA + nB
    nc = bacc.Bacc("TRN2", target_bir_lowering=False, debug=False)

    x_d = nc.declare_dram_parameter("x", [128, nch, ROWS], F16, isOutput=False)
    labs_d = nc.declare_dram_parameter("labs", [128, nch], F32, isOutput=False)
    iota_d = nc.declare_dram_parameter("iota", [128, 128], F32, isOutput=False)
    invca_d = nc.declare_dram_parameter("invca", [128, 1], F32, isOutput=False)
    invcb_d = nc.declare_dram_parameter("invcb", [RB, 1], F32, isOutput=False)
    i128_d = nc.declare_dram_parameter("i128", [128, 128], F32, isOutput=False)
    i72_d = nc.declare_dram_parameter("i72", [72, 72], F32, isOutput=False)
    out_d = nc.declare_dram_parameter("conn", [SPB, R, R], F32, isOutput=True)

    with tile.TileContext(nc) as tc:
        with tc.tile_pool(name="consts", bufs=1) as consts, \
             tc.tile_pool(name="loads", bufs=3) as loads, \
             tc.tile_pool(name="ohp", bufs=6) as ohp, \
             tc.tile_pool(name="epi", bufs=1) as epi, \
             tc.tile_pool(name="psum", bufs=1, space="PSUM") as psum:

            labs_s = consts.tile([128, nch], F32)
            iota_s = consts.tile([128, 128], F32)
            invca_s = consts.tile([128, 1], F32)
            invcb_s = consts.tile([RB, 1], F32)
            i128_s = consts.tile([128, 128], F32)
            i72_s = consts.tile([72, 72], F32)
            nc.sync.dma_start(labs_s[:], labs_d[:])
            nc.sync.dma_start(iota_s[:], iota_d[:])
            nc.sync.dma_start(invca_s[:], invca_d[:])
            nc.sync.dma_start(invcb_s[:], invcb_d[:])
            nc.sync.dma_start(i128_s[:], i128_d[:])
            nc.sync.dma_start(i72_s[:], i72_d[:])

            acc_a = psum.tile([128, ROWS], F32, tag="acc_a", bufs=1)
            acc_b = psum.tile([RB, ROWS], F32, tag="acc_b", bufs=1)

            with nc.named_scope("main"):
                ntiles = (nch + CT - 1) // CT
                for ti in range(ntiles):
                    ch0 = ti * CT
                    ct = min(CT, nch - ch0)
                    ld = loads.tile([128, ct, ROWS], F16, tag="ld", bufs=3,
                                    name=f"ld_{ti}")
                    eng = nc.scalar if (ti % 2 == 0) else nc.sync
                    eng.dma_start(ld[:], x_d[:, ch0:ch0 + ct, :])

                    for j in range(ct):
                        cc = ch0 + j
                        if cc < nA:
                            width, acc = 128, acc_a
                            start, stop = (cc == 0), (cc == nA - 1)
                        else:
                            width, acc = RB, acc_b
                            start, stop = (cc == nA), (cc == nch - 1)
                        oh = ohp.tile([128, width], F16, tag=f"oh{width}",
                                      bufs=6, name=f"oh_{cc}")
                        nc.vector.tensor_scalar(oh[:], iota_s[:, 0:width],
                                                labs_s[:, cc:cc + 1], None,
                                                op0=mybir.AluOpType.is_equal)
                        nc.tensor.matmul(acc[:], oh[:], ld[:, j, :],
                                         start=start, stop=stop)

            with nc.named_scope("epilogue"):
                # roi sums -> sbuf, scale by 1/count
                roi_a = epi.tile([128, ROWS], F32)
                roi_b = epi.tile([RB, ROWS], F32)
                nc.vector.tensor_copy(roi_a[:], acc_a[:])
                nc.vector.tensor_copy(roi_b[:], acc_b[:])
                nc.vector.tensor_scalar_mul(roi_a[:], roi_a[:], invca_s[:])
                nc.vector.tensor_scalar_mul(roi_b[:], roi_b[:], invcb_s[:])

                for s in range(SPB):
                    sl = bass.ts(s, T)
                    roiN = {}
                    for blk, rt, P in (("a", roi_a, 128), ("b", roi_b, RB)):
                        mean = epi.tile([P, 1], F32, name=f"mean_{blk}{s}",
                                        tag=f"mean_{blk}")
                        nc.vector.tensor_reduce(mean[:], rt[:, sl],
                                                axis=mybir.AxisListType.X,
                                                op=mybir.AluOpType.add)
                        nc.vector.tensor_scalar_mul(mean[:], mean[:], 1.0 / T)
                        rc = epi.tile([P, T], F32, name=f"rc_{blk}{s}",
                                      tag=f"rc_{blk}")
                        nc.vector.tensor_scalar(rc[:], rt[:, sl], mean[:], None,
                                                op0=mybir.AluOpType.subtract)
                        sq = epi.tile([P, T], F32, name=f"sq_{blk}{s}",
                                      tag=f"sq_{blk}")
                        ss = epi.tile([P, 1], F32, name=f"ss_{blk}{s}",
                                      tag=f"ss_{blk}")
                        nc.vector.scalar_tensor_tensor(
                            sq[:], rc[:], 1.0, rc[:],
                            op0=mybir.AluOpType.mult, op1=mybir.AluOpType.mult,
                            accum_out=ss[:])
                        nc.scalar.sqrt(ss[:], ss[:])
                        nc.vector.tensor_scalar_add(ss[:], ss[:], EPS)
                        nc.vector.reciprocal(ss[:], ss[:])
                        rn = epi.tile([P, T], F32, name=f"rn_{blk}{s}",
                                      tag=f"rn_{blk}")
                        nc.vector.tensor_scalar_mul(rn[:], rc[:], ss[:])
                        roiN[blk] = rn

                    # transpose roiN -> (t, r) on PE
                    trA = psum.tile([128, R], F32, tag="trA", bufs=1,
                                    name=f"trA_{s}")
                    trB = psum.tile([72, R], F32, tag="trB", bufs=1,
                                    name=f"trB_{s}")
                    nc.tensor.transpose(trA[:, 0:128], roiN["a"][:, 0:128], i128_s[:])
                    nc.tensor.transpose(trA[:, 128:200], roiN["b"][:, 0:128], i72_s[:])
                    nc.tensor.transpose(trB[:, 0:128], roiN["a"][:, 128:200], i128_s[:])
                    nc.tensor.transpose(trB[:, 128:200], roiN["b"][:, 128:200], i72_s[:])
                    trA_sb = epi.tile([128, R], F32, name=f"trAs_{s}", tag="trAs")
                    trB_sb = epi.tile([72, R], F32, name=f"trBs_{s}", tag="trBs")
                    nc.vector.tensor_copy(trA_sb[:], trA[:])
                    nc.vector.tensor_copy(trB_sb[:], trB[:])

                    # Gram: conn = roiN_t.T @ roiN_t  (contraction over t)
                    cA = psum.tile([128, R], F32, tag="cA", bufs=1, name=f"cA_{s}")
                    cB = psum.tile([72, R], F32, tag="cB", bufs=1, name=f"cB_{s}")
                    nc.tensor.matmul(cA[:], trA_sb[:, 0:128], trA_sb[:],
                                     start=True, stop=False)
                    nc.tensor.matmul(cA[:], trB_sb[:, 0:128], trB_sb[:],
                                     start=False, stop=True)
                    nc.tensor.matmul(cB[:], trA_sb[:, 128:200], trA_sb[:],
                                     start=True, stop=False)
                    nc.tensor.matmul(cB[:], trB_sb[:, 128:200], trB_sb[:],
                                     start=False, stop=True)
                    cA_sb = epi.tile([128, R], F32, name=f"cAs_{s}", tag="cAs")
                    cB_sb = epi.tile([72, R], F32, name=f"cBs_{s}", tag="cBs")
                    nc.vector.tensor_copy(cA_sb[:], cA[:])
                    nc.vector.tensor_copy(cB_sb[:], cB[:])
                    nc.sync.dma_start(out_d[s, 0:128, :], cA_sb[:])
                    nc.sync.dma_start(out_d[s, 128:200, :], cB_sb[:])

    nc.compile()
    return nc


def _get_program(nA, nB):
    key = (nA, nB)
    if key not in _cached:
        _cached[key] = _build_program(nA, nB)
    return _cached[key]


def marshal_inputs(x, parc, mask):
    """Host-side prep: packed ROI-sorted fp16 x + tiny derived constants."""
    parc_eff = np.where(np.asarray(mask), np.asarray(parc), 0).reshape(V)
    lab = parc_eff.astype(np.int64) - 1          # -1 = dropped
    counts = np.bincount(parc_eff.astype(np.int64), minlength=R + 1)[1:]
    inv = np.float32(1.0) / counts.astype(np.float32)

    order = np.argsort(lab, kind="stable")
    nbg = int((lab < 0).sum())
    sorted_idx = order[nbg:]                     # kept pixels, ROI-ascending
    cA = int(counts[0:RA].sum())
    cB = int(counts[RA:R].sum())
    nA = (cA + 127) // 128
    nB = (cB + 127) // 128

    gA = np.concatenate([sorted_idx[:cA],
                         np.zeros(nA * 128 - cA, dtype=np.int64)])
    gB = np.concatenate([sorted_idx[cA:],
                         np.zeros(nB * 128 - cB, dtype=np.int64)])
    g = np.concatenate([gA, gB])                 # (nch*128,) gather indices
    labA = np.concatenate([lab[sorted_idx[:cA]],
                           np.full(nA * 128 - cA, -1, dtype=np.int64)])
    labB = np.concatenate([lab[sorted_idx[cA:]] - RA,
                           np.full(nB * 128 - cB, -1, dtype=np.int64)])
    nch = nA + nB
    labs = np.concatenate([labA, labB]).astype(np.float32)
    labs = labs.reshape(nch, 128).T.copy()       # (128, nch)

    iota = np.broadcast_to(np.arange(128, dtype=np.float32), (128, 128)).copy()
    invca = inv[0:RA].reshape(RA, 1).copy()
    invcb = inv[RA:R].reshape(RB, 1).copy()
    i128 = np.eye(128, dtype=np.float32)
    i72 = np.eye(72, dtype=np.float32)

    # (N,1,T,H,W) fp32 -> packed (core, 128, nch, SPB*T) fp16
    x16 = np.asarray(x, dtype=np.float32).reshape(N, T, V).astype(np.float16)
    xg = x16[:, :, g]                            # (N, T, nch*128)
    xg = xg.reshape(NCORES, SPB, T, nch, 128)
    xs = np.ascontiguousarray(xg.transpose(0, 4, 3, 1, 2))  # (8,128,nch,2,T)
    xs = xs.reshape(NCORES, 128, nch, ROWS)

    in_maps = []
    for c in range(NCORES):
        in_maps.append({
            "x": xs[c], "labs": labs, "iota": iota,
            "invca": invca, "invcb": invcb, "i128": i128, "i72": i72,
        })
    return in_maps, nA, nB


def kernel(x, parc, mask):
    in_maps, nA, nB = marshal_inputs(x, parc, mask)
    nc = _get_program(nA, nB)
    res = run_bass_kernel_spmd(nc, in_maps, core_ids=list(range(NCORES)))
    conn = np.concatenate([r["conn"] for r in res.results], axis=0)  # (16,200,200)
    row, col = np.triu_indices(R, k=1)
    return np.ascontiguousarray(conn[:, row, col]).astype(np.float32)
